# revision 8
# baseline (speedup 1.0000x reference)
"""Causal self-attention (QAT fake-quant weights, RMS-normed q/k, RoPE, GQA)
on 8 Trainium2 NeuronCores.

Sharding: core c = b*4 + t  (b in {0,1} batch, t in {0..3} tensor-parallel).
Per core: 4 q-heads (t*4..t*4+3), 1 kv head (t), Wproj columns [512t, 512t+512).
Each core computes a full [D, S] transposed partial of the output projection;
the host transposes and sums the 4 TP partials per batch element.

Everything on-device is feature-major ("transposed"): activations [feat, seq].
 - projections:   qT = qWqT.T @ xT  (contraction over d on partitions)
 - scoresT[k,q]  = krotT_tile.T @ qrotT  -> exp -> probsT (SBUF, f32r)
 - PV:            yT += v_nat_tile.T @ probsT   (v natural = [s, hd])
 - softmax sums:  ones[128,1].T @ probsT -> [1, q] PSUM accumulation
 - out:           outT = qWPT.T @ (yT / sums)
RoPE rotate-half is a PE permutation matmul + DVE mul/adds; rms_norm sums of
squares are ones-matmuls over qT^2; gain and 1/sqrt(hd) fold into the rsqrt.
Softmax skips max-subtraction (scores bounded by gain*sqrt(hd) ~ 11.3).
Fake quant: round(W * (1/s)) * s with s = fp16(max|W_blk|/31) per 128-block,
rounding via the +1.5*2^23 magic-constant trick (RNE, matches jnp.round).
"""

import os
from contextlib import ExitStack

import numpy as np

import concourse.bass as bass
import concourse.bacc as bacc
import concourse.tile as tile
from concourse import mybir
from concourse.bass_utils import run_bass_kernel_spmd

F32 = mybir.dt.float32
F32R = mybir.dt.float32r
F16 = mybir.dt.float16

DIM = 2048
S = 2048
HD = 128
HL = 4            # local q heads per core
CL = HL * HD      # local head dims (proj contraction)
NB = DIM // 128   # 16 blocks of 128 along a full input-feature axis
MAGIC = float(1.5 * 2 ** 23)
INV31 = float(np.float32(1.0) / np.float32(31.0))
EPS = float(np.finfo(np.float32).eps)
F16_TINY = float(np.finfo(np.float16).tiny)

Alu = mybir.AluOpType
Act = mybir.ActivationFunctionType

_CACHE = {}


def _emit_quant_smalls(nc, pool, wn, nb, pfx):
    """wn [128, nb*128] natural weight tile -> (sf, rf): scale and 1/scale."""
    amax = pool.tile([128, nb], F32, tag=pfx + "am")
    nc.vector.tensor_reduce(
        amax[:], wn[:].rearrange("p (b c) -> p b c", c=128),
        axis=mybir.AxisListType.X, op=Alu.max, apply_absolute_value=True)
    s0 = pool.tile([128, nb], F32, tag=pfx + "s0")
    nc.vector.tensor_scalar(s0[:], amax[:], INV31, 1e-12, Alu.mult, Alu.max)
    s16 = pool.tile([128, nb], F16, tag=pfx + "s16")
    nc.vector.tensor_copy(s16[:], s0[:])
    s32 = pool.tile([128, nb], F32, tag=pfx + "s32")
    nc.vector.tensor_copy(s32[:], s16[:])
    sf = pool.tile([128, nb], F32, tag=pfx + "sf")
    nc.vector.tensor_scalar_max(sf[:], s32[:], F16_TINY)
    rf = pool.tile([128, nb], F32, tag=pfx + "rf")
    nc.vector.reciprocal(rf[:], sf[:])
    return sf, rf


def _emit_quant_apply(nc, wpool, wn, sf, rf, nb, qtag, ttag):
    """qw = round(wn * rf) * sf blockwise, via magic-constant RNE round."""
    qw = wpool.tile([128, nb * 128], F32, tag=qtag)
    tt = wpool.tile([128, nb * 128], F32, tag=ttag)
    for b in range(nb):
        sl = slice(b * 128, (b + 1) * 128)
        nc.vector.tensor_scalar(tt[:, sl], wn[:, sl], rf[:, b:b + 1], MAGIC,
                                Alu.mult, Alu.add)
        nc.vector.tensor_scalar(qw[:, sl], tt[:, sl], MAGIC, sf[:, b:b + 1],
                                Alu.subtract, Alu.mult)
    return qw


def build_nc():
    nc = bacc.Bacc("TRN2")

    XT = nc.dram_tensor("XT", [DIM, S], F32R, kind="ExternalInput")
    WQ = nc.dram_tensor("WQ", [CL, DIM], F32, kind="ExternalInput")
    WK = nc.dram_tensor("WK", [HD, DIM], F32, kind="ExternalInput")
    WV = nc.dram_tensor("WV", [HD, DIM], F32, kind="ExternalInput")
    WP = nc.dram_tensor("WP", [DIM, CL], F32, kind="ExternalInput")
    COSW = nc.dram_tensor("COSW", [128, S], F32, kind="ExternalInput")
    SINW = nc.dram_tensor("SINW", [128, S], F32, kind="ExternalInput")
    IDENT = nc.dram_tensor("IDENT", [128, 128], F32, kind="ExternalInput")
    PSWAP = nc.dram_tensor("PSWAP", [128, 128], F32R, kind="ExternalInput")
    TRIM = nc.dram_tensor("TRIM", [128, 128], F32, kind="ExternalInput")
    ONESC = nc.dram_tensor("ONESC", [128, 1], F32R, kind="ExternalInput")
    AVEC = nc.dram_tensor("AVEC", [1, 8], F32, kind="ExternalInput")
    BVEC = nc.dram_tensor("BVEC", [1, 8], F32, kind="ExternalInput")

    OUT = nc.dram_tensor("OUT", [DIM, S], F32, kind="ExternalOutput")

    copy_flip = [0]

    def copy_out(dst, src):
        # alternate PSUM->SBUF copies between ACT and DVE
        if copy_flip[0] % 2 == 0:
            nc.scalar.copy(dst, src)
        else:
            nc.vector.tensor_copy(dst, src)
        copy_flip[0] += 1

    with tile.TileContext(nc) as tc, ExitStack() as octx:
        # ---------------- always-live pools ----------------
        pc = octx.enter_context(tc.tile_pool(name="consts", bufs=1))
        prow = octx.enter_context(tc.tile_pool(name="rows", bufs=3))
        pdram = octx.enter_context(tc.tile_pool(name="dram", bufs=1,
                                                space="DRAM"))
        ps = octx.enter_context(tc.tile_pool(name="ps", bufs=4, space="PSUM"))
        psacc = octx.enter_context(tc.tile_pool(name="psacc", bufs=2,
                                                space="PSUM"))
        psrow = octx.enter_context(tc.tile_pool(name="psrow", bufs=2,
                                                space="PSUM"))

        ident = pc.tile([128, 128], F32)
        pswap = pc.tile([128, 128], F32R)
        trim = pc.tile([128, 128], F32)
        onesc = pc.tile([128, 1], F32R)
        avec = pc.tile([1, 8], F32)
        bvec = pc.tile([1, 8], F32)
        nc.sync.dma_start(ident[:], IDENT[:, :])
        nc.sync.dma_start(pswap[:], PSWAP[:, :])
        nc.sync.dma_start(trim[:], TRIM[:, :])
        nc.sync.dma_start(onesc[:], ONESC[:, :])
        nc.sync.dma_start(avec[:], AVEC[:, :])
        nc.sync.dma_start(bvec[:], BVEC[:, :])

        # yT spilled to DRAM between attention and output projection
        ytd = [pdram.tile([128, S], F32R, tag=f"ytd{h}", name=f"ytd{h}") for h in range(HL)]

        # ============== P1: quantize Wq/Wk/Wv + transpose ==============
        # qwt lives until the end; its 16 [128,512] tag slots are reused
        # for the quantized Wproj tiles in P5.
        pq1 = octx.enter_context(tc.tile_pool(name="qwt", bufs=1))
        qWqT = [pq1.tile([128, CL], F32R, tag=f"qwq{d}", name=f"qwq{d}")
                for d in range(NB)]
        qWkT = [pq1.tile([128, 4, 128], F32R, tag=f"qwk{g}", name=f"qwk{g}")
                for g in range(4)]
        qWvT = [pq1.tile([128, 4, 128], F32R, tag=f"qwv{g}", name=f"qwv{g}")
                for g in range(4)]

        es1 = ExitStack()   # P1 working pools — close right after P1
        pw2 = es1.enter_context(tc.tile_pool(name="p1w2", bufs=2))
        pw4 = es1.enter_context(tc.tile_pool(name="p1w4", bufs=4))
        pws = es1.enter_context(tc.tile_pool(name="p1s", bufs=2))

        for W, dst in ((WK, qWkT), (WV, qWvT)):
            wn = pw2.tile([128, DIM], F32, tag="wnat")
            nc.sync.dma_start(wn[:], W[:, :])
            sf, rf = _emit_quant_smalls(nc, pws, wn, NB, "q")
            qw = _emit_quant_apply(nc, pw4, wn, sf, rf, NB, "wqq", "wtmp")
            for g in range(4):
                pt = ps.tile([128, 512], F32, tag="mm")
                for j in range(4):
                    blk = 4 * g + j
                    nc.tensor.transpose(pt[:, j * 128:(j + 1) * 128],
                                        qw[:, blk * 128:(blk + 1) * 128],
                                        ident[:])
                copy_out(dst[g][:].rearrange("p a b -> p (a b)"), pt[:])

        # Wq: 4 natural row-tiles; keep the 4 qw tiles for batched transposes
        qwq = []
        for ot in range(4):
            wn = pw2.tile([128, DIM], F32, tag="wnat")
            nc.sync.dma_start(wn[:], WQ[ot * 128:(ot + 1) * 128, :])
            sf, rf = _emit_quant_smalls(nc, pws, wn, NB, "q")
            qwq.append(_emit_quant_apply(nc, pw4, wn, sf, rf, NB,
                                         "wqq", "wtmp"))
        for blk in range(NB):
            pt = ps.tile([128, 512], F32, tag="mm")
            for ot in range(4):
                nc.tensor.transpose(pt[:, ot * 128:(ot + 1) * 128],
                                    qwq[ot][:, blk * 128:(blk + 1) * 128],
                                    ident[:])
            copy_out(qWqT[blk][:], pt[:])

        es1.close()

        # persistent attention operands (allocated after P1 pools freed)
        pp = octx.enter_context(tc.tile_pool(name="persist", bufs=1))
        qrot = [pp.tile([128, S], F32R, tag=f"qrot{h}", name=f"qrot{h}")
                for h in range(HL)]
        krot = pp.tile([128, S], F32R, tag="krot")
        vnat = pp.tile([128, NB, 128], F32R, tag="vnat")  # [s%128, s//128, hd]

        # ============== P2+P3 fused: projections + rms + rope =========
        es2 = ExitStack()
        px = es2.enter_context(tc.tile_pool(name="p2x", bufs=20))
        p2t = es2.enter_context(tc.tile_pool(name="p2t", bufs=2))
        p2c = es2.enter_context(tc.tile_pool(name="p2c", bufs=7))
        p2b = es2.enter_context(tc.tile_pool(name="p2b", bufs=2))

        for sc in range(4):
            ssl = slice(sc * 512, (sc + 1) * 512)
            xts = []
            for dt in range(NB):
                xt = px.tile([128, 512], F32R, tag="xt")
                nc.sync.dma_start(xt[:], XT[dt * 128:(dt + 1) * 128, ssl])
                xts.append(xt)
            cosw = p2t.tile([128, 512], F32, tag="cosw")
            sinw = p2t.tile([128, 512], F32, tag="sinw")
            nc.sync.dma_start(cosw[:], COSW[:, ssl])
            nc.sync.dma_start(sinw[:], SINW[:, ssl])

            raws = []
            for hm in range(HL + 1):  # 4 q heads then k
                pm = ps.tile([128, 512], F32, tag="mm")
                for dt in range(NB):
                    if hm < HL:
                        lhs = qWqT[dt][:, hm * 128:(hm + 1) * 128]
                    else:
                        lhs = qWkT[dt // 4][:, dt % 4, :]
                    nc.tensor.matmul(pm[:], lhs, xts[dt][:],
                                     start=(dt == 0), stop=(dt == NB - 1))
                raw = p2c.tile([128, 512], F32, tag="raw")
                nc.scalar.copy(raw[:], pm[:])
                raws.append(raw)
            # v projection; transpose to natural [s, hd]
            pm = ps.tile([128, 512], F32, tag="mm")
            for dt in range(NB):
                nc.tensor.matmul(pm[:], qWvT[dt // 4][:, dt % 4, :],
                                 xts[dt][:], start=(dt == 0),
                                 stop=(dt == NB - 1))
            vtr = p2c.tile([128, 512], F32, tag="raw")
            nc.scalar.copy(vtr[:], pm[:])
            pv = ps.tile([128, 512], F32, tag="mm")
            for j in range(4):
                nc.tensor.transpose(pv[:, j * 128:(j + 1) * 128],
                                    vtr[:, j * 128:(j + 1) * 128], ident[:])
            nc.vector.tensor_copy(
                vnat[:, 4 * sc:4 * sc + 4, :].rearrange("p a b -> p (a b)"),
                pv[:])

            for hm in range(HL + 1):
                raw = raws[hm]
                dst = qrot[hm] if hm < HL else krot
                qsq = p2t.tile([128, 512], F32R, tag="qsq")
                nc.vector.tensor_mul(qsq[:], raw[:], raw[:])
                ssp = psrow.tile([1, 512], F32, tag="row")
                nc.tensor.matmul(ssp[:], onesc[:], qsq[:],
                                 start=True, stop=True)
                pre = prow.tile([1, 512], F32, tag="prerow")
                nc.scalar.activation(pre[:], ssp[:], Act.Ln,
                                     bias=bvec[0:1, hm:hm + 1],
                                     scale=avec[0:1, hm:hm + 1])
                rr = prow.tile([1, 512], F32, tag="prerow")
                nc.scalar.activation(rr[:], pre[:], Act.Exp, scale=-0.5)
                rb = p2b.tile([128, 512], F32, tag="rb")
                nc.gpsimd.partition_broadcast(rb[:], rr[:])
                qn = p2t.tile([128, 512], F32R, tag="qn")
                nc.vector.tensor_mul(qn[:], raw[:], rb[:])
                # rope: dst = qn*cos + (PSWAP @ qn)*sin
                sw = ps.tile([128, 512], F32, tag="mm")
                nc.tensor.matmul(sw[:], pswap[:], qn[:],
                                 start=True, stop=True)
                u = p2t.tile([128, 512], F32, tag="u")
                nc.vector.tensor_mul(u[:], qn[:], cosw[:])
                w = p2t.tile([128, 512], F32, tag="w")
                nc.vector.tensor_mul(w[:], sw[:], sinw[:])
                nc.vector.tensor_add(dst[:, ssl], u[:], w[:])
        es2.close()

        # ============== P5 (Wproj quant) + P4 (attention) + P6 ==============
        es3 = ExitStack()
        p5w = es3.enter_context(tc.tile_pool(name="p5w", bufs=4))
        p5s = es3.enter_context(tc.tile_pool(name="p5s", bufs=2))
        pprob = es3.enter_context(tc.tile_pool(name="probs", bufs=8))
        pm4 = es3.enter_context(tc.tile_pool(name="p4m", bufs=2))

        qWPT = [pq1.tile([128, 512], F32R, tag=f"qwq{i}", name=f"qwp{i}")
                for i in range(16)]
        for og in range(4):  # groups of 4 o-tiles
            qwps = []
            for j in range(4):
                ot = 4 * og + j
                wn = p5w.tile([128, CL], F32, tag="wnat5")
                nc.sync.dma_start(wn[:], WP[ot * 128:(ot + 1) * 128, :])
                sf, rf = _emit_quant_smalls(nc, p5s, wn, 4, "p")
                qwps.append(_emit_quant_apply(nc, p5w, wn, sf, rf, 4,
                                              "wqp", "wtp"))
            for blk in range(4):
                pt = ps.tile([128, 512], F32, tag="mm")
                for j in range(4):
                    nc.tensor.transpose(
                        pt[:, j * 128:(j + 1) * 128],
                        qwps[j][:, blk * 128:(blk + 1) * 128], ident[:])
                copy_out(qWPT[4 * blk + og][:], pt[:])

        # ---- P4: attention ----
        for h in range(HL):
            qr = qrot[h]
            for qc in range(4):
                qsl = slice(qc * 512, (qc + 1) * 512)
                yps = psacc.tile([128, 512], F32, tag="acc")
                sps = psrow.tile([1, 512], F32, tag="row")
                nkt = 4 * qc + 4
                for kt in range(nkt):
                    j = kt - 4 * qc
                    lo = 0 if j < 0 else 128 * j
                    scp = ps.tile([128, 512], F32, tag="mm")
                    nc.tensor.matmul(
                        scp[:, lo:], krot[:, kt * 128:(kt + 1) * 128],
                        qr[:, qc * 512 + lo:(qc + 1) * 512],
                        start=True, stop=True)
                    pr = pprob.tile([128, 512], F32R, tag="pr")
                    nc.scalar.activation(pr[:, lo:], scp[:, lo:], Act.Exp)
                    if j >= 0:
                        nc.vector.tensor_mul(pr[:, lo:lo + 128],
                                             pr[:, lo:lo + 128], trim[:])
                    nc.tensor.matmul(yps[:, lo:], vnat[:, kt, :], pr[:, lo:],
                                     start=(kt == 0), stop=(kt == nkt - 1))
                    nc.tensor.matmul(sps[0:1, lo:], onesc[:], pr[:, lo:],
                                     start=(kt == 0), stop=(kt == nkt - 1))
                lnm = prow.tile([1, 512], F32, tag="prerow")
                nc.scalar.activation(lnm[:], sps[:], Act.Ln)
                rs = prow.tile([1, 512], F32, tag="prerow")
                nc.scalar.activation(rs[:], lnm[:], Act.Exp, scale=-1.0)
                rb2 = pm4.tile([128, 512], F32, tag="rb2")
                nc.gpsimd.partition_broadcast(rb2[:], rs[:])
                ya = pm4.tile([128, 512], F32, tag="ya")
                nc.scalar.copy(ya[:], yps[:])
                yt = pm4.tile([128, 512], F32R, tag="yt")
                nc.vector.tensor_mul(yt[:], ya[:], rb2[:])
                nc.sync.dma_start(ytd[h][:, qsl], yt[:])

        # ---- P6: output projection ----
        p6y = es3.enter_context(tc.tile_pool(name="p6y", bufs=8))
        p6o = es3.enter_context(tc.tile_pool(name="p6o", bufs=3))
        for qc in range(4):
            qsl = slice(qc * 512, (qc + 1) * 512)
            yts = []
            for hb in range(HL):
                yti = p6y.tile([128, 512], F32R, tag="ytin")
                nc.sync.dma_start(yti[:], ytd[hb][:, qsl])
                yts.append(yti)
            for ot in range(NB):
                op = ps.tile([128, 512], F32, tag="mm")
                for blk in range(4):
                    lhs = qWPT[4 * blk + ot // 4][:, (ot % 4) * 128:
                                                  (ot % 4 + 1) * 128]
                    nc.tensor.matmul(op[:], lhs, yts[blk][:],
                                     start=(blk == 0), stop=(blk == 3))
                ob = p6o.tile([128, 512], F32, tag="ob")
                copy_out(ob[:], op[:])
                nc.sync.dma_start(OUT[ot * 128:(ot + 1) * 128, qsl], ob[:])
        es3.close()

    nc.compile()
    return nc


# --------------------------------------------------------------------------
# host side
# --------------------------------------------------------------------------

def _host_consts():
    inv_freq = 1.0 / (10000.0 ** (np.arange(0, HD, 2, dtype=np.float32)
                                  / np.float32(HD)))
    freqs = np.outer(np.arange(S, dtype=np.float32),
                     inv_freq).astype(np.float32)       # [S, 64]
    cosT = np.cos(freqs).astype(np.float32).T           # [64, S]
    sinT = np.sin(freqs).astype(np.float32).T
    cosw = np.ascontiguousarray(np.concatenate([cosT, cosT], axis=0))
    sinw = np.ascontiguousarray(np.concatenate([sinT, -sinT], axis=0))
    ident = np.eye(128, dtype=np.float32)
    pswap = np.zeros((128, 128), dtype=np.float32)
    pswap[:64, 64:] = np.eye(64)
    pswap[64:, :64] = np.eye(64)
    trim = (np.arange(128)[:, None] <= np.arange(128)[None, :]) \
        .astype(np.float32)                             # allow k <= q
    onesc = np.ones((128, 1), dtype=np.float32)
    return cosw, sinw, ident, pswap, trim, onesc


def kernel(x, Wq, Wk, Wv, Wproj, q_gain):
    x = np.asarray(x, dtype=np.float32)
    Wq = np.asarray(Wq, dtype=np.float32)
    Wk = np.asarray(Wk, dtype=np.float32)
    Wv = np.asarray(Wv, dtype=np.float32)
    Wproj = np.asarray(Wproj, dtype=np.float32)
    q_gain = np.asarray(q_gain, dtype=np.float32)
    B = x.shape[0]

    if "nc" not in _CACHE:
        _CACHE["nc"] = build_nc()
    nc = _CACHE["nc"]

    cosw, sinw, ident, pswap, trim, onesc = _host_consts()

    in_maps = []
    for c in range(8):
        b, t = divmod(c, 4)
        g = q_gain[4 * t:4 * t + 4].astype(np.float64)
        avec = np.zeros((1, 8), dtype=np.float32)
        bvec = np.zeros((1, 8), dtype=np.float32)
        avec[0, :4] = (1.0 / g ** 2).astype(np.float32)
        avec[0, 4] = np.float32(1.0 / 128.0)
        bvec[0, :4] = (128.0 * EPS / g ** 2).astype(np.float32)
        bvec[0, 4] = np.float32(EPS)
        in_maps.append({
            "XT": np.ascontiguousarray(x[b].T),
            "WQ": np.ascontiguousarray(Wq[CL * t:CL * (t + 1), :]),
            "WK": np.ascontiguousarray(Wk[HD * t:HD * (t + 1), :]),
            "WV": np.ascontiguousarray(Wv[HD * t:HD * (t + 1), :]),
            "WP": np.ascontiguousarray(Wproj[:, CL * t:CL * (t + 1)]),
            "COSW": cosw, "SINW": sinw, "IDENT": ident, "PSWAP": pswap,
            "TRIM": trim, "ONESC": onesc, "AVEC": avec, "BVEC": bvec,
        })

    res = run_bass_kernel_spmd(
        nc, in_maps, core_ids=list(range(8)),
        trace=bool(int(os.environ.get("KERNEL_TRACE", "0"))))
    _CACHE["last_results"] = res

    out = np.zeros((B, S, DIM), dtype=np.float32)
    for c in range(8):
        b = c // 4
        out[b] += res.results[c]["OUT"].T
    return out


# revision 9
# speedup vs baseline: 1.3905x; 1.3905x over previous
"""Causal self-attention (QAT fake-quant weights, RMS-normed q/k, RoPE, GQA)
on 8 Trainium2 NeuronCores.

Sharding: core c = b*4 + t  (b in {0,1} batch, t in {0..3} tensor-parallel).
Per core: 4 q-heads (t*4..t*4+3), 1 kv head (t), Wproj columns [512t, 512t+512).
Each core computes a full [D, S] transposed partial of the output projection;
the host transposes and sums the 4 TP partials per batch element.

Everything on-device is feature-major ("transposed"): activations [feat, seq].
 - projections:   qT = qWqT.T @ xT  (contraction over d on partitions)
 - scoresT[k,q]  = krotT_tile.T @ qrotT  -> exp -> probsT (SBUF, f32r)
 - PV:            yT += v_nat_tile.T @ probsT   (v natural = [s, hd])
 - softmax sums:  ones[128,1].T @ probsT -> [1, q] PSUM accumulation
 - out:           outT = qWPT.T @ (yT / sums)
RoPE rotate-half is a PE permutation matmul + DVE mul/adds; rms_norm sums of
squares are ones-matmuls over qT^2; gain and 1/sqrt(hd) fold into the rsqrt.
Softmax skips max-subtraction (scores bounded by gain*sqrt(hd) ~ 11.3).
Fake quant: round(W * (1/s)) * s with s = fp16(max|W_blk|/31) per 128-block,
rounding via the +1.5*2^23 magic-constant trick (RNE, matches jnp.round).
"""

import os
from contextlib import ExitStack

import numpy as np

import concourse.bass as bass
import concourse.bacc as bacc
import concourse.tile as tile
from concourse import mybir
from concourse.bass_utils import run_bass_kernel_spmd

F32 = mybir.dt.float32
F32R = mybir.dt.float32r
F16 = mybir.dt.float16

DIM = 2048
S = 2048
HD = 128
HL = 4            # local q heads per core
CL = HL * HD      # local head dims (proj contraction)
NB = DIM // 128   # 16 blocks of 128 along a full input-feature axis
MAGIC = float(1.5 * 2 ** 23)
INV31 = float(np.float32(1.0) / np.float32(31.0))
EPS = float(np.finfo(np.float32).eps)
F16_TINY = float(np.finfo(np.float16).tiny)

Alu = mybir.AluOpType
Act = mybir.ActivationFunctionType

_CACHE = {}


def _emit_quant_smalls(nc, pool, wn, nb, pfx):
    """wn [128, nb*128] natural weight tile -> (sf, rf): scale and 1/scale."""
    amax = pool.tile([128, nb], F32, tag=pfx + "am")
    nc.vector.tensor_reduce(
        amax[:], wn[:].rearrange("p (b c) -> p b c", c=128),
        axis=mybir.AxisListType.X, op=Alu.max, apply_absolute_value=True)
    s0 = pool.tile([128, nb], F32, tag=pfx + "s0")
    nc.vector.tensor_scalar(s0[:], amax[:], INV31, 1e-12, Alu.mult, Alu.max)
    s16 = pool.tile([128, nb], F16, tag=pfx + "s16")
    nc.vector.tensor_copy(s16[:], s0[:])
    s32 = pool.tile([128, nb], F32, tag=pfx + "s32")
    nc.vector.tensor_copy(s32[:], s16[:])
    sf = pool.tile([128, nb], F32, tag=pfx + "sf")
    nc.vector.tensor_scalar_max(sf[:], s32[:], F16_TINY)
    rf = pool.tile([128, nb], F32, tag=pfx + "rf")
    nc.vector.reciprocal(rf[:], sf[:])
    return sf, rf


def _emit_quant_apply(nc, wpool, wn, sf, rf, nb, qtag, ttag):
    """qw = round(wn * rf) * sf blockwise, via magic-constant RNE round."""
    qw = wpool.tile([128, nb * 128], F32, tag=qtag)
    tt = wpool.tile([128, nb * 128], F32, tag=ttag)
    for b in range(nb):
        sl = slice(b * 128, (b + 1) * 128)
        nc.vector.tensor_scalar(tt[:, sl], wn[:, sl], rf[:, b:b + 1], MAGIC,
                                Alu.mult, Alu.add)
        nc.vector.tensor_scalar(qw[:, sl], tt[:, sl], MAGIC, sf[:, b:b + 1],
                                Alu.subtract, Alu.mult)
    return qw


def build_nc():
    nc = bacc.Bacc("TRN2")

    XT = nc.dram_tensor("XT", [DIM, S], F32R, kind="ExternalInput")
    WQ = nc.dram_tensor("WQ", [CL, DIM], F32, kind="ExternalInput")
    WK = nc.dram_tensor("WK", [HD, DIM], F32, kind="ExternalInput")
    WV = nc.dram_tensor("WV", [HD, DIM], F32, kind="ExternalInput")
    WP = nc.dram_tensor("WP", [DIM, CL], F32, kind="ExternalInput")
    COSW = nc.dram_tensor("COSW", [128, S], F32, kind="ExternalInput")
    SINW = nc.dram_tensor("SINW", [128, S], F32, kind="ExternalInput")
    IDENT = nc.dram_tensor("IDENT", [128, 128], F32, kind="ExternalInput")
    PSWAP = nc.dram_tensor("PSWAP", [128, 128], F32R, kind="ExternalInput")
    TRIM = nc.dram_tensor("TRIM", [128, 128], F32, kind="ExternalInput")
    ONESC = nc.dram_tensor("ONESC", [128, 1], F32R, kind="ExternalInput")
    AVEC = nc.dram_tensor("AVEC", [1, 8], F32, kind="ExternalInput")
    BVEC = nc.dram_tensor("BVEC", [1, 8], F32, kind="ExternalInput")

    OUT = nc.dram_tensor("OUT", [DIM, S], F32, kind="ExternalOutput")

    copy_flip = [0]

    def copy_out(dst, src):
        # alternate PSUM->SBUF copies between ACT and DVE
        if copy_flip[0] % 2 == 0:
            nc.scalar.copy(dst, src)
        else:
            nc.vector.tensor_copy(dst, src)
        copy_flip[0] += 1

    with tile.TileContext(nc) as tc, ExitStack() as octx:
        # ---------------- always-live pools ----------------
        pc = octx.enter_context(tc.tile_pool(name="consts", bufs=1))
        prow = octx.enter_context(tc.tile_pool(name="rows", bufs=3))
        pdram = octx.enter_context(tc.tile_pool(name="dram", bufs=1,
                                                space="DRAM"))
        ps = octx.enter_context(tc.tile_pool(name="ps", bufs=4, space="PSUM"))
        psacc = octx.enter_context(tc.tile_pool(name="psacc", bufs=2,
                                                space="PSUM"))
        psrow = octx.enter_context(tc.tile_pool(name="psrow", bufs=2,
                                                space="PSUM"))

        ident = pc.tile([128, 128], F32)
        pswap = pc.tile([128, 128], F32R)
        trim = pc.tile([128, 128], F32)
        onesc = pc.tile([128, 1], F32R)
        avec = pc.tile([1, 8], F32)
        bvec = pc.tile([1, 8], F32)
        nc.sync.dma_start(ident[:], IDENT[:, :])
        nc.sync.dma_start(pswap[:], PSWAP[:, :])
        nc.sync.dma_start(trim[:], TRIM[:, :])
        nc.sync.dma_start(onesc[:], ONESC[:, :])
        nc.sync.dma_start(avec[:], AVEC[:, :])
        nc.sync.dma_start(bvec[:], BVEC[:, :])

        # yT spilled to DRAM between attention and output projection
        ytd = [pdram.tile([128, S], F32R, tag=f"ytd{h}", name=f"ytd{h}") for h in range(HL)]

        # ============== P1: quantize Wq/Wk/Wv + transpose ==============
        # qwt lives until the end; its 16 [128,512] tag slots are reused
        # for the quantized Wproj tiles in P5.
        pq1 = octx.enter_context(tc.tile_pool(name="qwt", bufs=1))
        qWqT = [pq1.tile([128, CL], F32R, tag=f"qwq{d}", name=f"qwq{d}")
                for d in range(NB)]
        qWkT = [pq1.tile([128, 4, 128], F32R, tag=f"qwk{g}", name=f"qwk{g}")
                for g in range(4)]
        qWvT = [pq1.tile([128, 4, 128], F32R, tag=f"qwv{g}", name=f"qwv{g}")
                for g in range(4)]

        es1 = ExitStack()   # P1 working pools — close right after P1
        pw2 = es1.enter_context(tc.tile_pool(name="p1w2", bufs=2))
        pw4 = es1.enter_context(tc.tile_pool(name="p1w4", bufs=4))
        pws = es1.enter_context(tc.tile_pool(name="p1s", bufs=2))

        for W, dst in ((WK, qWkT), (WV, qWvT)):
            wn = pw2.tile([128, DIM], F32, tag="wnat")
            nc.sync.dma_start(wn[:], W[:, :])
            sf, rf = _emit_quant_smalls(nc, pws, wn, NB, "q")
            qw = _emit_quant_apply(nc, pw4, wn, sf, rf, NB, "wqq", "wtmp")
            for g in range(4):
                pt = ps.tile([128, 512], F32, tag="mm")
                for j in range(4):
                    blk = 4 * g + j
                    nc.tensor.transpose(pt[:, j * 128:(j + 1) * 128],
                                        qw[:, blk * 128:(blk + 1) * 128],
                                        ident[:])
                copy_out(dst[g][:].rearrange("p a b -> p (a b)"), pt[:])

        # Wq: 4 natural row-tiles; keep the 4 qw tiles for batched transposes
        qwq = []
        for ot in range(4):
            wn = pw2.tile([128, DIM], F32, tag="wnat")
            nc.sync.dma_start(wn[:], WQ[ot * 128:(ot + 1) * 128, :])
            sf, rf = _emit_quant_smalls(nc, pws, wn, NB, "q")
            qwq.append(_emit_quant_apply(nc, pw4, wn, sf, rf, NB,
                                         "wqq", "wtmp"))
        for blk in range(NB):
            pt = ps.tile([128, 512], F32, tag="mm")
            for ot in range(4):
                nc.tensor.transpose(pt[:, ot * 128:(ot + 1) * 128],
                                    qwq[ot][:, blk * 128:(blk + 1) * 128],
                                    ident[:])
            copy_out(qWqT[blk][:], pt[:])

        es1.close()

        # persistent attention operands (allocated after P1 pools freed)
        pp = octx.enter_context(tc.tile_pool(name="persist", bufs=1))
        qrot = [pp.tile([128, S], F32R, tag=f"qrot{h}", name=f"qrot{h}")
                for h in range(HL)]
        krot = pp.tile([128, S], F32R, tag="krot")
        vnat = pp.tile([128, NB, 128], F32R, tag="vnat")  # [s%128, s//128, hd]

        # ============== P2+P3 fused: projections + rms + rope =========
        es2 = ExitStack()
        px = es2.enter_context(tc.tile_pool(name="p2x", bufs=20))
        p2t = es2.enter_context(tc.tile_pool(name="p2t", bufs=2))
        p2c = es2.enter_context(tc.tile_pool(name="p2c", bufs=7))
        p2b = es2.enter_context(tc.tile_pool(name="p2b", bufs=2))

        for sc in range(4):
            ssl = slice(sc * 512, (sc + 1) * 512)
            xts = []
            for dt in range(NB):
                xt = px.tile([128, 512], F32R, tag="xt")
                nc.sync.dma_start(xt[:], XT[dt * 128:(dt + 1) * 128, ssl])
                xts.append(xt)
            cosw = p2t.tile([128, 512], F32, tag="cosw")
            sinw = p2t.tile([128, 512], F32, tag="sinw")
            nc.sync.dma_start(cosw[:], COSW[:, ssl])
            nc.sync.dma_start(sinw[:], SINW[:, ssl])

            raws = []
            for hm in range(HL + 1):  # 4 q heads then k
                pm = ps.tile([128, 512], F32, tag="mm")
                for dt in range(NB):
                    if hm < HL:
                        lhs = qWqT[dt][:, hm * 128:(hm + 1) * 128]
                    else:
                        lhs = qWkT[dt // 4][:, dt % 4, :]
                    nc.tensor.matmul(pm[:], lhs, xts[dt][:],
                                     start=(dt == 0), stop=(dt == NB - 1))
                raw = p2c.tile([128, 512], F32, tag="raw")
                nc.scalar.copy(raw[:], pm[:])
                raws.append(raw)
            # v projection; transpose to natural [s, hd]
            pm = ps.tile([128, 512], F32, tag="mm")
            for dt in range(NB):
                nc.tensor.matmul(pm[:], qWvT[dt // 4][:, dt % 4, :],
                                 xts[dt][:], start=(dt == 0),
                                 stop=(dt == NB - 1))
            vtr = p2c.tile([128, 512], F32, tag="raw")
            nc.scalar.copy(vtr[:], pm[:])
            pv = ps.tile([128, 512], F32, tag="mm")
            for j in range(4):
                nc.tensor.transpose(pv[:, j * 128:(j + 1) * 128],
                                    vtr[:, j * 128:(j + 1) * 128], ident[:])
            nc.vector.tensor_copy(
                vnat[:, 4 * sc:4 * sc + 4, :].rearrange("p a b -> p (a b)"),
                pv[:])

            for hm in range(HL + 1):
                raw = raws[hm]
                dst = qrot[hm] if hm < HL else krot
                qsq = p2t.tile([128, 512], F32R, tag="qsq")
                nc.vector.tensor_mul(qsq[:], raw[:], raw[:])
                ssp = psrow.tile([1, 512], F32, tag="row")
                nc.tensor.matmul(ssp[:], onesc[:], qsq[:],
                                 start=True, stop=True)
                rr = prow.tile([1, 512], F32, tag="prerow")
                nc.scalar.activation(rr[:], ssp[:], Act.Abs_reciprocal_sqrt,
                                     bias=bvec[0:1, hm:hm + 1],
                                     scale=avec[0:1, hm:hm + 1])
                rb = p2b.tile([128, 512], F32, tag="rb")
                nc.gpsimd.partition_broadcast(rb[:], rr[:])
                qn = p2t.tile([128, 512], F32R, tag="qn")
                nc.vector.tensor_mul(qn[:], raw[:], rb[:])
                # rope: dst = qn*cos + (PSWAP @ qn)*sin
                sw = ps.tile([128, 512], F32, tag="mm")
                nc.tensor.matmul(sw[:], pswap[:], qn[:],
                                 start=True, stop=True)
                u = p2t.tile([128, 512], F32, tag="u")
                nc.vector.tensor_mul(u[:], qn[:], cosw[:])
                w = p2t.tile([128, 512], F32, tag="w")
                nc.vector.tensor_mul(w[:], sw[:], sinw[:])
                nc.vector.tensor_add(dst[:, ssl], u[:], w[:])
        es2.close()

        # ============== P5 (Wproj quant) + P4 (attention) + P6 ==============
        es3 = ExitStack()
        p5w = es3.enter_context(tc.tile_pool(name="p5w", bufs=4))
        p5s = es3.enter_context(tc.tile_pool(name="p5s", bufs=2))
        pprob = es3.enter_context(tc.tile_pool(name="probs", bufs=8))
        pm4 = es3.enter_context(tc.tile_pool(name="p4m", bufs=2))

        qWPT = [pq1.tile([128, 512], F32R, tag=f"qwq{i}", name=f"qwp{i}")
                for i in range(16)]
        for og in range(4):  # groups of 4 o-tiles
            qwps = []
            for j in range(4):
                ot = 4 * og + j
                wn = p5w.tile([128, CL], F32, tag="wnat5")
                nc.sync.dma_start(wn[:], WP[ot * 128:(ot + 1) * 128, :])
                sf, rf = _emit_quant_smalls(nc, p5s, wn, 4, "p")
                qwps.append(_emit_quant_apply(nc, p5w, wn, sf, rf, 4,
                                              "wqp", "wtp"))
            for blk in range(4):
                pt = ps.tile([128, 512], F32, tag="mm")
                for j in range(4):
                    nc.tensor.transpose(
                        pt[:, j * 128:(j + 1) * 128],
                        qwps[j][:, blk * 128:(blk + 1) * 128], ident[:])
                copy_out(qWPT[4 * blk + og][:], pt[:])

        # ---- P4: attention ----
        for h in range(HL):
            qr = qrot[h]
            for qc in range(4):
                qsl = slice(qc * 512, (qc + 1) * 512)
                yps = psacc.tile([128, 512], F32, tag="acc")
                sps = psrow.tile([1, 512], F32, tag="row")
                nkt = 4 * qc + 4
                for kt in range(nkt):
                    j = kt - 4 * qc
                    lo = 0 if j < 0 else 128 * j
                    scp = ps.tile([128, 512], F32, tag="mm")
                    nc.tensor.matmul(
                        scp[:, lo:], krot[:, kt * 128:(kt + 1) * 128],
                        qr[:, qc * 512 + lo:(qc + 1) * 512],
                        start=True, stop=True)
                    pr = pprob.tile([128, 512], F32R, tag="pr")
                    nc.scalar.activation(pr[:, lo:], scp[:, lo:], Act.Exp)
                    if j >= 0:
                        nc.vector.tensor_mul(pr[:, lo:lo + 128],
                                             pr[:, lo:lo + 128], trim[:])
                    nc.tensor.matmul(yps[:, lo:], vnat[:, kt, :], pr[:, lo:],
                                     start=(kt == 0), stop=(kt == nkt - 1))
                    nc.tensor.matmul(sps[0:1, lo:], onesc[:], pr[:, lo:],
                                     start=(kt == 0), stop=(kt == nkt - 1))
                scr = prow.tile([1, 512], F32, tag="prerow")
                rs = prow.tile([1, 512], F32, tag="prerow")
                nc.vector.reciprocal_approx_accurate(rs[:], sps[:], scr[:])
                rb2 = pm4.tile([128, 512], F32, tag="rb2")
                nc.gpsimd.partition_broadcast(rb2[:], rs[:])
                ya = pm4.tile([128, 512], F32, tag="ya")
                nc.scalar.copy(ya[:], yps[:])
                yt = pm4.tile([128, 512], F32R, tag="yt")
                nc.vector.tensor_mul(yt[:], ya[:], rb2[:])
                nc.sync.dma_start(ytd[h][:, qsl], yt[:])

        # ---- P6: output projection ----
        p6y = es3.enter_context(tc.tile_pool(name="p6y", bufs=8))
        p6o = es3.enter_context(tc.tile_pool(name="p6o", bufs=3))
        for qc in range(4):
            qsl = slice(qc * 512, (qc + 1) * 512)
            yts = []
            for hb in range(HL):
                yti = p6y.tile([128, 512], F32R, tag="ytin")
                nc.sync.dma_start(yti[:], ytd[hb][:, qsl])
                yts.append(yti)
            for ot in range(NB):
                op = ps.tile([128, 512], F32, tag="mm")
                for blk in range(4):
                    lhs = qWPT[4 * blk + ot // 4][:, (ot % 4) * 128:
                                                  (ot % 4 + 1) * 128]
                    nc.tensor.matmul(op[:], lhs, yts[blk][:],
                                     start=(blk == 0), stop=(blk == 3))
                ob = p6o.tile([128, 512], F32, tag="ob")
                copy_out(ob[:], op[:])
                nc.sync.dma_start(OUT[ot * 128:(ot + 1) * 128, qsl], ob[:])
        es3.close()

    nc.compile()
    return nc


# --------------------------------------------------------------------------
# host side
# --------------------------------------------------------------------------

def _host_consts():
    inv_freq = 1.0 / (10000.0 ** (np.arange(0, HD, 2, dtype=np.float32)
                                  / np.float32(HD)))
    freqs = np.outer(np.arange(S, dtype=np.float32),
                     inv_freq).astype(np.float32)       # [S, 64]
    cosT = np.cos(freqs).astype(np.float32).T           # [64, S]
    sinT = np.sin(freqs).astype(np.float32).T
    cosw = np.ascontiguousarray(np.concatenate([cosT, cosT], axis=0))
    sinw = np.ascontiguousarray(np.concatenate([sinT, -sinT], axis=0))
    ident = np.eye(128, dtype=np.float32)
    pswap = np.zeros((128, 128), dtype=np.float32)
    pswap[:64, 64:] = np.eye(64)
    pswap[64:, :64] = np.eye(64)
    trim = (np.arange(128)[:, None] <= np.arange(128)[None, :]) \
        .astype(np.float32)                             # allow k <= q
    onesc = np.ones((128, 1), dtype=np.float32)
    return cosw, sinw, ident, pswap, trim, onesc


def kernel(x, Wq, Wk, Wv, Wproj, q_gain):
    x = np.asarray(x, dtype=np.float32)
    Wq = np.asarray(Wq, dtype=np.float32)
    Wk = np.asarray(Wk, dtype=np.float32)
    Wv = np.asarray(Wv, dtype=np.float32)
    Wproj = np.asarray(Wproj, dtype=np.float32)
    q_gain = np.asarray(q_gain, dtype=np.float32)
    B = x.shape[0]

    if "nc" not in _CACHE:
        _CACHE["nc"] = build_nc()
    nc = _CACHE["nc"]

    cosw, sinw, ident, pswap, trim, onesc = _host_consts()

    in_maps = []
    for c in range(8):
        b, t = divmod(c, 4)
        g = q_gain[4 * t:4 * t + 4].astype(np.float64)
        avec = np.zeros((1, 8), dtype=np.float32)
        bvec = np.zeros((1, 8), dtype=np.float32)
        avec[0, :4] = (1.0 / g ** 2).astype(np.float32)
        avec[0, 4] = np.float32(1.0 / 128.0)
        bvec[0, :4] = (128.0 * EPS / g ** 2).astype(np.float32)
        bvec[0, 4] = np.float32(EPS)
        in_maps.append({
            "XT": np.ascontiguousarray(x[b].T),
            "WQ": np.ascontiguousarray(Wq[CL * t:CL * (t + 1), :]),
            "WK": np.ascontiguousarray(Wk[HD * t:HD * (t + 1), :]),
            "WV": np.ascontiguousarray(Wv[HD * t:HD * (t + 1), :]),
            "WP": np.ascontiguousarray(Wproj[:, CL * t:CL * (t + 1)]),
            "COSW": cosw, "SINW": sinw, "IDENT": ident, "PSWAP": pswap,
            "TRIM": trim, "ONESC": onesc, "AVEC": avec, "BVEC": bvec,
        })

    res = run_bass_kernel_spmd(
        nc, in_maps, core_ids=list(range(8)),
        trace=bool(int(os.environ.get("KERNEL_TRACE", "0"))))
    _CACHE["last_results"] = res

    out = np.zeros((B, S, DIM), dtype=np.float32)
    for c in range(8):
        b = c // 4
        out[b] += res.results[c]["OUT"].T
    return out
